# revision 1
# baseline (speedup 1.0000x reference)
"""Trainium2 Bass kernel for nn_CBAMSLayer: spatial-attention CBAM block.

Reference computation (per full input x [32, 256, 56, 56]):
    y  = stack([max_c(x), mean_c(x)])          # [N, 2, H, W]
    y  = conv5x5(y, conv_w)                    # [N, 1, H, W], SAME pad
    y  = batchnorm_train(y, gamma, beta)       # stats over (N, H, W)
    out = x * sigmoid(sigmoid(y))

Sharding: data-parallel over batch, 4 images per core on 8 cores; BN batch
statistics (sum, sumsq of y) are all-reduced across cores.

Per-core layout strategy (all engine ops at partition base 0):
  - x shard kept resident in SBUF as 8 tiles [128, 3136] (c-half x image).
  - PE transposes 112-wide hw blocks of both c-halves into PSUM
    [112 hw, 2x128 c]; DVE reduce-max and ScalarE accum (channel sum) produce
    the conv inputs directly in "partition space" [112=(h2,col), n, b] where
    hw = b*112 + h2*56 + col (b = row-pair index, h2 = row parity).
  - The 5x5 conv becomes 6 accumulated matmuls with host-precomputed
    112x112 matrices (3 row-pair shifts x 2 channels), fed via `wmat` input.
  - BN stats: ScalarE accum_out + 112->1 matmul fold; AllReduce [1,16];
    scale/bias broadcast to partitions via DMA; double sigmoid on ScalarE.
  - Gate returned to row form via one PE transpose + flatten DMA, then
    replicated across 128 partitions with K=1 matmuls; DVE multiplies the
    resident x tiles in place; DMA out.
"""
import numpy as np

NCORES = 8
NIMG = 4
C = 256
HW = 3136
NB = 28          # hw blocks per image
BW = 112         # block width (2 rows of 56)
EPS = 1e-5
TOTAL_COUNT = NCORES * NIMG * HW

_cache = {}


def _make_wmat(conv_w):
    """6 GEMM matrices [p_in, p_out] for (ch, db): y += W^T @ C[:, :, b+db]."""
    wk = np.asarray(conv_w, np.float64).reshape(2, 5, 5).copy()
    wk[1] /= C  # fold mean = sum/C into the weights of the mean channel
    Wm = np.zeros((2, 3, 112, 112), np.float64)
    for h2 in (0, 1):
        for c in range(56):
            for sr in (-2, -1, 0, 1, 2):
                h2p = (h2 + sr) % 2
                db = (h2 + sr - h2p) // 2
                for sc in (-2, -1, 0, 1, 2):
                    cp = c + sc
                    if 0 <= cp < 56:
                        for ch in range(2):
                            Wm[ch, db + 1, h2p * 56 + cp, h2 * 56 + c] += wk[ch, sr + 2, sc + 2]
    # order i = ch*3 + (db+1); layout [p_in, i*112 + p_out]
    return np.ascontiguousarray(
        Wm.reshape(6, 112, 112).transpose(1, 0, 2).reshape(112, 672)
    ).astype(np.float32)


def _build(gamma, beta):
    import concourse.bass as bass
    import concourse.bacc as bacc
    import concourse.tile as tile
    from concourse import mybir, masks
    from contextlib import ExitStack

    F32 = mybir.dt.float32
    AX = mybir.AxisListType
    OP = mybir.AluOpType
    ACT = mybir.ActivationFunctionType

    nc = bacc.Bacc("TRN2", target_bir_lowering=False, debug=False, num_devices=NCORES)
    x = nc.dram_tensor("x", [NIMG, C, HW], F32, kind="ExternalInput").ap()
    wm = nc.dram_tensor("wmat", [112, 672], F32, kind="ExternalInput").ap()
    out = nc.dram_tensor("out", [NIMG, C, HW], F32, kind="ExternalOutput").ap()
    cc_in = nc.dram_tensor("cc_in", [1, 16], F32).ap()
    cc_out = nc.dram_tensor("cc_out", [1, 16], F32, addr_space="Shared").ap()

    with tile.TileContext(nc) as tc, ExitStack() as ctx:
        sb = ctx.enter_context(tc.tile_pool(name="sb", bufs=1))
        mp = ctx.enter_context(tc.tile_pool(name="mp", bufs=2))
        srp = ctx.enter_context(tc.tile_pool(name="srp", bufs=2))
        sfp = ctx.enter_context(tc.tile_pool(name="sfp", bufs=2))

        X = [[sb.tile([128, HW], F32, tag=f"x{n}h{h}", name=f"x{n}h{h}") for h in range(2)]
             for n in range(NIMG)]
        for n in range(NIMG):
            nc.sync.dma_start(out=X[n][0][:], in_=x[n, 0:128, :])
            nc.sync.dma_start(out=X[n][1][:], in_=x[n, 128:256, :])

        Wt = sb.tile([112, 672], F32)
        nc.sync.dma_start(out=Wt[:], in_=wm)

        ident = sb.tile([128, 128], F32)
        masks.make_identity(nc, ident[:])

        Cmx = sb.tile([112, NIMG, 30], F32)
        Csm = sb.tile([112, NIMG, 30], F32)
        nc.gpsimd.memset(Cmx[:], 0.0)
        nc.gpsimd.memset(Csm[:], 0.0)
        scol = sb.tile([112, 2], F32)
        ysb = sb.tile([112, NIMG, NB], F32)
        strash2 = sb.tile([112, 112], F32)
        s1 = sb.tile([112, NIMG, NB], F32)
        s2 = sb.tile([112, 112], F32)
        sTs = sb.tile([112, 112], F32)
        ones128 = sb.tile([128, 1], F32)
        nc.vector.memset(ones128[:], 1.0)
        ones112 = sb.tile([112, 1], F32)
        ocol = sb.tile([1, 128], F32)
        nc.vector.memset(ones112[:], 1.0)
        nc.vector.memset(ocol[:], 1.0)
        eps_t = sb.tile([112, 1], F32)
        nc.vector.memset(eps_t[:], EPS)
        stats_bc = sb.tile([112, 2], F32)
        mean_t = sb.tile([112, 1], F32)
        e2_t = sb.tile([112, 1], F32)
        var_t = sb.tile([112, 1], F32)
        sd_t = sb.tile([112, 1], F32)
        rstd_t = sb.tile([112, 1], F32)
        scale_t = sb.tile([112, 1], F32)
        bias_t = sb.tile([112, 1], F32)
        st_sb = sb.tile([1, 16], F32)

        with ExitStack() as p2:
            tp = p2.enter_context(tc.tile_pool(name="tp", bufs=3, space="PSUM"))
            sp = p2.enter_context(tc.tile_pool(name="sp", bufs=2, space="PSUM"))
            pyp = p2.enter_context(tc.tile_pool(name="pyp", bufs=1, space="PSUM"))
            pfp = p2.enter_context(tc.tile_pool(name="pfp", bufs=1, space="PSUM"))
            stp = p2.enter_context(tc.tile_pool(name="stp", bufs=1, space="PSUM"))

            # ---- channel max: fold halves on DVE, transpose M, reduce ----
            for n in range(NIMG):
                M = mp.tile([128, HW], F32, tag="m", name="M")
                nc.vector.tensor_tensor(out=M[:], in0=X[n][0][:],
                                        in1=X[n][1][:], op=OP.max)
                for t in range(NB // 4):
                    pt = tp.tile([112, 4, 128], F32, tag="tp", name="pt")
                    for blk in range(4):
                        b = 4 * t + blk
                        nc.tensor.matmul(
                            pt[:, blk, :],
                            M[:, b * BW:(b + 1) * BW],
                            ident[:],
                            is_transpose=True,
                            start=True, stop=True,
                            skip_group_check=True,
                        )
                    nc.vector.tensor_reduce(
                        out=Cmx[:, n, 1 + 4 * t:5 + 4 * t], in_=pt[:],
                        axis=AX.X, op=OP.max)

                # ---- channel sum: matmul-ones into p-outer psum rows ----
                srow = srp.tile([1, HW], F32, tag="srow", name="srow")
                for k in range(7):
                    sp_t = sp.tile([1, 448], F32, tag="sp", name="sp_t")
                    for h in range(2):
                        nc.tensor.matmul(sp_t[:], ones128[:],
                                         X[n][h][:, k * 448:(k + 1) * 448],
                                         start=(h == 0), stop=(h == 1),
                                         skip_group_check=True)
                    # permute chunk to p-outer order during the PSUM->SBUF copy:
                    # srow[p*28 + 4k + b'] = sp_t[b'*112 + p]
                    nc.scalar.copy(
                        srow.rearrange("q (p k b) -> q k b p", k=7, b=4)[:, k],
                        sp_t[:])
                nc.sync.dma_start(
                    out=Csm[:, n, 1:29],
                    in_=srow.rearrange("q (p b) -> q p b", b=28))

            # ---- conv as 6 accumulated matmuls ----
            yp = pyp.tile([112, NIMG, NB], F32)
            i = 0
            for Ct in (Cmx, Csm):
                for db in (-1, 0, 1):
                    nc.tensor.matmul(
                        yp[:], Wt[:, i * 112:(i + 1) * 112],
                        Ct[:, :, 1 + db:29 + db],
                        start=(i == 0), stop=(i == 5),
                        skip_group_check=True)
                    i += 1

            # ---- BN stats + all-reduce ----
            nc.scalar.activation(out=ysb[:], in_=yp[:], func=ACT.Copy,
                                 accum_out=scol[:, 0:1])
            nc.scalar.activation(out=strash2[:],
                                 in_=ysb.rearrange("p n b -> p (n b)"),
                                 func=ACT.Square, accum_out=scol[:, 1:2])
            pf = pfp.tile([1, 2], F32)
            nc.tensor.matmul(pf[0:1, :], ones112[:], scol[:], start=True, stop=True)
            nc.vector.memset(st_sb[:], 0.0)
            nc.scalar.copy(st_sb[:, 0:2], pf[0:1, :])
            nc.sync.dma_start(out=cc_in, in_=st_sb[:])
            nc.gpsimd.collective_compute(
                "AllReduce", OP.add,
                replica_groups=[list(range(NCORES))],
                ins=[cc_in], outs=[cc_out])
            bcast = bass.AP(tensor=cc_out.tensor, offset=cc_out.offset,
                            ap=[[0, 112], [1, 2]])
            nc.gpsimd.dma_start(out=stats_bc[:], in_=bcast)

            # ---- BN scale/bias (per-partition copies of global scalars) ----
            inv = 1.0 / TOTAL_COUNT
            nc.vector.tensor_scalar_mul(mean_t[:], stats_bc[:, 0:1], inv)
            nc.vector.tensor_scalar_mul(e2_t[:], stats_bc[:, 1:2], inv)
            nc.vector.tensor_scalar(out=var_t[:], in0=mean_t[:],
                                    scalar1=mean_t[:], scalar2=-1.0,
                                    op0=OP.mult, op1=OP.mult)
            nc.vector.tensor_tensor(out=var_t[:], in0=var_t[:], in1=e2_t[:],
                                    op=OP.add)
            nc.scalar.activation(out=sd_t[:], in_=var_t[:], func=ACT.Sqrt,
                                 bias=eps_t[:])
            nc.vector.reciprocal(rstd_t[:], sd_t[:])
            nc.vector.tensor_scalar_mul(scale_t[:], rstd_t[:], float(gamma))
            nc.vector.tensor_scalar(out=bias_t[:], in0=mean_t[:],
                                    scalar1=scale_t[:], scalar2=-1.0,
                                    op0=OP.mult, op1=OP.mult)
            if float(beta) != 0.0:
                nc.vector.tensor_scalar_add(bias_t[:], bias_t[:], float(beta))

            # ---- gate: sigmoid(sigmoid(scale*y + bias)) ----
            nc.scalar.activation(out=s1[:], in_=ysb[:], func=ACT.Sigmoid,
                                 bias=bias_t[:], scale=scale_t[:])
            nc.scalar.activation(out=s2[:],
                                 in_=s1.rearrange("p n b -> p (n b)"),
                                 func=ACT.Sigmoid)

            # ---- gate to row form ----
            sT = stp.tile([112, 112], F32)
            nc.tensor.matmul(sT[:], s2[:], ident[0:112, 0:112],
                             is_transpose=True, start=True, stop=True,
                             skip_group_check=True)
            nc.scalar.copy(sTs[:], sT[:])

        # ---- stage D: out = x * gate (gate replicated over partitions) ----
        with ExitStack() as p3:
            dp = p3.enter_context(tc.tile_pool(name="dp", bufs=2, space="PSUM"))
            for n in range(NIMG):
                sflat = sfp.tile([1, HW], F32, tag="sf", name="sflat")
                nc.sync.dma_start(
                    out=sflat.rearrange("q (p f) -> q p f", p=112),
                    in_=sTs[n * 28:(n + 1) * 28, :])
                for half in range(2):
                    c0 = half * 1568
                    dt = dp.tile([128, 1568], F32, tag="d", name="dt")
                    for o0, cw in ((0, 512), (512, 512), (1024, 512), (1536, 32)):
                        nc.tensor.matmul(
                            dt[:, o0:o0 + cw], ocol[:],
                            sflat[0:1, c0 + o0:c0 + o0 + cw],
                            start=True, stop=True, skip_group_check=True)
                    for h in range(2):
                        nc.vector.tensor_tensor(
                            out=X[n][h][:, c0:c0 + 1568],
                            in0=X[n][h][:, c0:c0 + 1568],
                            in1=dt[:], op=OP.mult)
                nc.sync.dma_start(out=out[n, 0:128, :], in_=X[n][0][:])
                nc.sync.dma_start(out=out[n, 128:256, :], in_=X[n][1][:])

    nc.compile()
    return nc


def _get_nc(gamma, beta):
    key = (round(float(gamma), 9), round(float(beta), 9))
    if key not in _cache:
        _cache[key] = _build(float(gamma), float(beta))
    return _cache[key]


def kernel(x, conv_w, gamma, beta):
    from concourse.bass_utils import run_bass_kernel_spmd

    x = np.asarray(x, np.float32)
    conv_w = np.asarray(conv_w, np.float32)
    g = float(np.asarray(gamma).reshape(-1)[0])
    b = float(np.asarray(beta).reshape(-1)[0])

    xs = np.ascontiguousarray(x.reshape(NCORES, NIMG, C, HW))
    wmat = _make_wmat(conv_w)

    nc = _get_nc(g, b)
    in_maps = [{"x": xs[i], "wmat": wmat} for i in range(NCORES)]
    res = run_bass_kernel_spmd(nc, in_maps, list(range(NCORES))).results
    o = np.stack([res[i]["out"] for i in range(NCORES)], axis=0)
    return o.reshape(NCORES * NIMG, C, 56, 56)

